# revision 36
# baseline (speedup 1.0000x reference)
"""Trainium2 Bass kernel for nn_AttentionHead (B=4, S=4096, D_IN=1024, DK=DV=64).

Sharding: 8 cores = batch(4) x query-half(2). Each core computes attention for
its 2048 query rows against the full 4096-key sequence of its batch.

Host prep: inputs are cast to bf16 and transposed to [D_IN, seq] on the host,
so the device does plain contiguous HWDGE loads (no swizzle DMA, no on-chip
stream transpose, half the HBM bytes).

Per-core device pipeline (flash-style streaming over kv granules of 4 chunks):
  1. Loads: granule tiles [128, 8, cols] bf16 via nc.sync DMA, double buffered.
  2. Projections with W stationary, W column-duplicated so PSUM rows 64-127
     hold a copy: one [128, 512] eviction (bias add) writes both the base and
     the high-partition copy used for PE row-tile packing.
  3. Scores: row-tiled pairs — chunk 2c on PE rows 0-63, chunk 2c+1 on rows
     64-127, concurrent, N=512 each, into a [128, 1024] PSUM tile.
  4. Exp on ScalarE in N=1024 blocks (f32 PSUM -> bf16 SBUF), double buffered.
  5. Softmax denominator: running per-partition sums dacc[qb] += ex on
     DVE (qb 0/1) and GpSimd (qb 2/3); final 128-partition reduce via tiny
     ones-matmuls at the end.
  6. PV: col-tiled pairs — qb pair (a,b) share one PSUM bank, M=64 each at
     output partitions 0-63 / 64-127, concurrent, accumulated over all 32
     kv chunks.
  7. Finalize: transpose [128, 128] blocks (two qb at once), per-partition
     scale by 1/denom, one batched store.
"""
import os
import numpy as np
import ml_dtypes

import concourse.bass as bass
import concourse.mybir as mybir
import concourse.tile as tile
from concourse import bacc
from concourse.bass_utils import run_bass_kernel_spmd
from concourse.masks import make_identity

F32 = mybir.dt.float32
BF16 = mybir.dt.bfloat16
EXP = mybir.ActivationFunctionType.Exp
NPBF16 = ml_dtypes.bfloat16

B, S, D_IN, DK, DV = 4, 4096, 1024, 64, 64
SQ = S // 2            # 2048 query rows per core
NCH = D_IN // 128      # 8 d_in chunks
NKV = S // 128         # 32 kv chunks of 128
NQB = SQ // 512        # 4 query blocks of 512
KVG = 512              # kv granule column width (4 chunks)
NG = S // KVG          # 8 kv granules

_NC_CACHE = {}


def build_attention_nc():
    nc = bacc.Bacc()

    # host-preblocked granules: [g, p, c, s] = xT[c*128+p, g*GW+s]
    qt_ext = nc.declare_dram_parameter("qt", [4, 128, NCH, KVG], BF16, isOutput=False)
    kt_ext = nc.declare_dram_parameter("kt", [NG, 128, NCH, KVG], BF16, isOutput=False)
    vt_ext = nc.declare_dram_parameter("vt", [NG, 128, NCH, KVG], BF16, isOutput=False)
    # host-prepacked weights (bf16, Wq/Wk column-duplicated) and biases (dup)
    wq_ext = nc.declare_dram_parameter("wq", [128, NCH, 128], BF16, isOutput=False)
    wk_ext = nc.declare_dram_parameter("wk", [128, NCH, 128], BF16, isOutput=False)
    wv_ext = nc.declare_dram_parameter("wv", [128, NCH, DV], BF16, isOutput=False)
    bias_ext = nc.declare_dram_parameter("bias", [128, 3], F32, isOutput=False)
    out_ext = nc.declare_dram_parameter("out", [SQ, DV], F32, isOutput=True)

    with tile.TileContext(nc) as tc:
        with (
            tc.tile_pool(name="sg", bufs=1) as sg,
            tc.tile_pool(name="src", bufs=4) as srcp,
            tc.tile_pool(name="exp", bufs=6) as expp,
            tc.tile_pool(name="fin", bufs=2) as fin,
            tc.tile_pool(name="pp", bufs=2, space="PSUM") as pp,
            tc.tile_pool(name="sc", bufs=2, space="PSUM") as scp,
            tc.tile_pool(name="ot", bufs=2, space="PSUM") as otp,
        ):
            # ---- weights/biases first (small, prepacked), then streaming loads,
            # all on the single sync HWDGE ring so arrival order == issue order
            Wq = sg.tile([128, NCH, 128], BF16)
            Wk = sg.tile([128, NCH, 128], BF16)
            Wv = sg.tile([128, NCH, DV], BF16)
            biases = sg.tile([128, 3], F32)
            nc.sync.dma_start(out=Wq[:, :, :], in_=wq_ext[:, :, :])
            nc.sync.dma_start(out=biases[:, :], in_=bias_ext[:, :])

            src_tiles = {}

            def load(kind, idx):
                ext = {"q": qt_ext, "k": kt_ext, "v": vt_ext}[kind]
                t = srcp.tile(
                    [128, NCH, KVG], BF16, tag=f"{kind}src", bufs=3,
                    name=f"src_{kind}{idx}",
                )
                nc.sync.dma_start(out=t[:, :, :], in_=ext[idx])
                src_tiles[(kind, idx)] = t

            load("q", 0)
            nc.sync.dma_start(out=Wk[:, :, :], in_=wk_ext[:, :, :])
            load("k", 0)
            nc.sync.dma_start(out=Wv[:, :, :], in_=wv_ext[:, :, :])
            load("v", 0)
            load("q", 1)
            load("q", 2)
            load("q", 3)
            for g in range(1, NG):
                load("k", g)
                load("v", g)

            # ---- constants
            identb = sg.tile([128, 128], BF16)
            make_identity(nc, identb[:, :])
            identf = sg.tile([128, 128], F32)
            make_identity(nc, identf[:, :])
            ones = sg.tile([128, 1], BF16)
            nc.vector.memset(ones[:, :], 1.0)
            bqd = biases[:, 0:1]
            bkd = biases[:, 1:2]
            bvd = biases[0:64, 2:3]

            # projected tensors
            qTd = sg.tile([128, SQ], BF16)   # rows 0-63 = qT, 64-127 = copy
            kTd = sg.tile([128, S], BF16)    # rows 0-63 = kT, 64-127 = copy
            vT = sg.tile([64, S], BF16)      # [dv, kv]
            v1 = sg.tile([128, NKV, DV], BF16)  # v natural per chunk
            dacc = sg.tile([128, NQB, 1024], BF16)  # partial softmax denominators

            # prime PE clock and keep HAM warm while first loads land
            prime_ps = pp.tile([128, 128], BF16, tag="pp")
            for _ in range(8):
                nc.tensor.transpose(prime_ps[:, :], identb[:, :], identb[:, :])

            # PV accumulators: one bank per qb pair, col-tiled M=64 each
            otAB = otp.tile([128, 512], F32, tag="ot", name="otAB")
            otCD = otp.tile([128, 512], F32, tag="ot", name="otCD")

            def project(kind, idx):
                """Project one 512-col granule; evict with bias into qTd/kTd/vT."""
                src = src_tiles.pop((kind, idx))
                W = {"q": Wq, "k": Wk, "v": Wv}[kind]
                col0 = KVG * idx
                mdim = 128 if kind != "v" else 64
                ps = pp.tile([128, 512], F32, tag="pp", name=f"pp_{kind}{idx}")
                for c in range(NCH):
                    nc.tensor.matmul(
                        ps[0:mdim, :],
                        W[:, c, 0:mdim],
                        src[:, c, :],
                        start=(c == 0),
                        stop=(c == NCH - 1),
                    )
                if kind == "q":
                    nc.scalar.add(qTd[:, col0 : col0 + 512], ps[:, :], bqd)
                elif kind == "k":
                    nc.scalar.add(kTd[:, col0 : col0 + 512], ps[:, :], bkd)
                else:
                    nc.scalar.add(vT[:, col0 : col0 + 512], ps[0:64, :], bvd)

            def vflip(c):
                """vT chunk c -> v1[:, c, :] (natural [kv, dv])."""
                ps = pp.tile([128, DV], BF16, tag="pp", name=f"vf{c}")
                nc.tensor.transpose(
                    ps[:, :], vT[:, 128 * c : 128 * (c + 1)], identb[0:64, 0:64]
                )
                nc.vector.tensor_copy(v1[:, c, :], ps[:, :])

            def attn_pair(p, step):
                """Scores+exp+denom+PV for chunk pair (2p, 2p+1), all qb."""
                exs = {}
                for qb in range(NQB):
                    sps = scp.tile([128, 1024], F32, tag="sc", name=f"sc{p}_{qb}")
                    for j in range(2):
                        c = 2 * p + j
                        # alternate row-half per slot so the next slot's
                        # LDWEIGHTS targets the idle row group and pulls ahead
                        lo = 64 * ((j + qb) % 2)
                        hi = lo + 64
                        nc.tensor.matmul(
                            sps[:, 512 * j : 512 * j + 512],
                            kTd[lo:hi, 128 * c : 128 * (c + 1)],
                            qTd[lo:hi, 512 * qb : 512 * qb + 512],
                            start=True,
                            stop=True,
                        )
                    ex = expp.tile([128, 1024], BF16, tag="ex", name=f"ex{p}_{qb}")
                    nc.scalar.activation(out=ex[:, :], in_=sps[:, :], func=EXP, scale=0.125)
                    exs[qb] = ex
                    eng = nc.vector if qb < 2 else nc.gpsimd
                    if p == 0:
                        eng.tensor_copy(dacc[:, qb, :], ex[:, :])
                    else:
                        eng.tensor_add(dacc[:, qb, :], dacc[:, qb, :], ex[:, :])
                    if qb == 1:
                        for j in range(2):
                            c = 2 * p + j
                            for half, qa in ((0, 0), (64, 1)):
                                nc.tensor.matmul(
                                    otAB[half : half + 64, :],
                                    v1[:, c, :],
                                    exs[qa][:, 512 * j : 512 * j + 512],
                                    start=(c == 0),
                                    stop=(c == NKV - 1),
                                )
                    step()
                for j in range(2):
                    c = 2 * p + j
                    for half, qa in ((0, 2), (64, 3)):
                        nc.tensor.matmul(
                            otCD[half : half + 64, :],
                            v1[:, c, :],
                            exs[qa][:, 512 * j : 512 * j + 512],
                            start=(c == 0),
                            stop=(c == NKV - 1),
                        )

            # ---- prologue projections
            project("q", 0)
            project("k", 0)
            project("v", 0)
            for c in range(4):
                vflip(c)
            project("q", 1)

            # ---- main streaming loop over kv granules
            for g in range(NG):
                work = []
                if g == 0:
                    work = [lambda: project("q", 2), lambda: project("q", 3)]
                if g + 1 < NG:
                    work += [
                        lambda g=g: project("k", g + 1),
                        lambda g=g: project("v", g + 1),
                    ] + [
                        (lambda g=g, i=i: vflip(4 * (g + 1) + i)) for i in range(4)
                    ]
                it = iter(work)

                def step(it=it):
                    nxt = next(it, None)
                    if nxt is not None:
                        nxt()

                attn_pair(2 * g, step)
                attn_pair(2 * g + 1, step)
                step()
                step()

            # ---- epilogue: denominators, normalize, store
            dn = pp.tile([128, 16], F32, tag="pp", name="dn")
            for qb in range(NQB):
                for t in range(4):
                    for h in range(2):
                        nc.tensor.matmul(
                            dn[:, 4 * qb + t : 4 * qb + t + 1],
                            dacc[:, qb, 512 * h + 128 * t : 512 * h + 128 * t + 128],
                            ones[:, :],
                            start=(h == 0),
                            stop=(h == 1),
                        )
            rd = fin.tile([128, 16], F32, tag="rd")
            nc.vector.reciprocal(rd[:, :], dn[:, :])

            osbAB = fin.tile([128, 512], F32, tag="osb", name="osbAB")
            osbCD = fin.tile([128, 512], F32, tag="osb", name="osbCD")
            nc.vector.tensor_copy(osbAB[:, :], otAB[:, :])
            nc.scalar.copy(osbCD[:, :], otCD[:, :])

            stage = sg.tile([128, 16, DV], F32)
            for pair, osb in ((0, osbAB), (1, osbCD)):
                for t in range(4):
                    tp = pp.tile([128, 128], F32, tag="pp", name=f"tp{pair}_{t}")
                    nc.tensor.transpose(
                        tp[:, :], osb[:, 128 * t : 128 * t + 128], identf[:, :]
                    )
                    for h in range(2):
                        qb = 2 * pair + h
                        if h == 0:
                            nc.scalar.activation(
                                out=stage[:, 4 * qb + t, :],
                                in_=tp[:, 0:64],
                                func=mybir.ActivationFunctionType.Identity,
                                scale=rd[:, 4 * qb + t : 4 * qb + t + 1],
                            )
                        else:
                            nc.vector.tensor_scalar_mul(
                                stage[:, 4 * qb + t, :],
                                tp[:, 64 : 128],
                                rd[:, 4 * qb + t : 4 * qb + t + 1],
                            )
            nc.sync.dma_start(
                out=out_ext.rearrange("(b p) n -> p b n", p=128), in_=stage[:, :, :]
            )

    nc.compile()
    return nc


def _get_nc():
    if "nc" not in _NC_CACHE:
        _NC_CACHE["nc"] = build_attention_nc()
    return _NC_CACHE["nc"]


def kernel(query, key, value, Wq, bq, Wk, bk, Wv, bv):
    query = np.asarray(query, dtype=np.float32)
    key = np.asarray(key, dtype=np.float32)
    value = np.asarray(value, dtype=np.float32)
    wq = np.ascontiguousarray(np.asarray(Wq, np.float32))
    wk = np.ascontiguousarray(np.asarray(Wk, np.float32))
    wv = np.ascontiguousarray(np.asarray(Wv, np.float32))
    bq_ = np.ascontiguousarray(np.asarray(bq, np.float32))
    bk_ = np.ascontiguousarray(np.asarray(bk, np.float32))
    bv_ = np.ascontiguousarray(np.asarray(bv, np.float32))

    def preblock(x, gw):
        # x: [rows, 1024] f32 -> [rows/gw, 128, 8, gw] bf16 with
        # out[g, p, c, s] = x[g*gw + s, c*128 + p]
        ng = x.shape[0] // gw
        return np.ascontiguousarray(
            x.astype(NPBF16).reshape(ng, gw, NCH, 128).transpose(0, 3, 2, 1)
        )

    def packw(w, dup):
        # [1024, 64] f32 -> [128, 8, 128 or 64] bf16, [p, c, :] = w[c*128+p, :]
        w8 = w.astype(NPBF16).reshape(NCH, 128, 64).transpose(1, 0, 2)
        if dup:
            w8 = np.concatenate([w8, w8], axis=2)
        return np.ascontiguousarray(w8)

    wqp = packw(wq, True)
    wkp = packw(wk, True)
    wvp = packw(wv, False)
    biasp = np.ascontiguousarray(
        np.stack(
            [np.tile(bq_, 2), np.tile(bk_, 2), np.tile(bv_, 2)], axis=1
        ).astype(np.float32)
    )

    in_maps = []
    for b in range(B):
        ktb = preblock(key[b], KVG)
        vtb = preblock(value[b], KVG)
        for h in range(2):
            qtb = preblock(query[b, h * SQ : (h + 1) * SQ], KVG)
            in_maps.append(
                {
                    "qt": qtb, "kt": ktb, "vt": vtb,
                    "wq": wqp, "wk": wkp, "wv": wvp, "bias": biasp,
                }
            )

    nc = _get_nc()
    trace = bool(int(os.environ.get("BASS_KERNEL_TRACE", "0")))
    res = run_bass_kernel_spmd(nc, in_maps, core_ids=list(range(8)), trace=trace)
    _NC_CACHE["last_results"] = res

    out = np.empty((B, S, DV), np.float32)
    for b in range(B):
        for h in range(2):
            out[b, h * SQ : (h + 1) * SQ] = res.results[2 * b + h]["out"]
    return out


# revision 37
# speedup vs baseline: 1.0076x; 1.0076x over previous
"""Trainium2 Bass kernel for nn_AttentionHead (B=4, S=4096, D_IN=1024, DK=DV=64).

Sharding: 8 cores = batch(4) x query-half(2). Each core computes attention for
its 2048 query rows against the full 4096-key sequence of its batch.

Host prep: inputs are cast to bf16 and transposed to [D_IN, seq] on the host,
so the device does plain contiguous HWDGE loads (no swizzle DMA, no on-chip
stream transpose, half the HBM bytes).

Per-core device pipeline (flash-style streaming over kv granules of 4 chunks):
  1. Loads: granule tiles [128, 8, cols] bf16 via nc.sync DMA, double buffered.
  2. Projections with W stationary, W column-duplicated so PSUM rows 64-127
     hold a copy: one [128, 512] eviction (bias add) writes both the base and
     the high-partition copy used for PE row-tile packing.
  3. Scores: row-tiled pairs — chunk 2c on PE rows 0-63, chunk 2c+1 on rows
     64-127, concurrent, N=512 each, into a [128, 1024] PSUM tile.
  4. Exp on ScalarE in N=1024 blocks (f32 PSUM -> bf16 SBUF), double buffered.
  5. Softmax denominator: running per-partition sums dacc[qb] += ex on
     DVE (qb 0/1) and GpSimd (qb 2/3); final 128-partition reduce via tiny
     ones-matmuls at the end.
  6. PV: col-tiled pairs — qb pair (a,b) share one PSUM bank, M=64 each at
     output partitions 0-63 / 64-127, concurrent, accumulated over all 32
     kv chunks.
  7. Finalize: transpose [128, 128] blocks (two qb at once), per-partition
     scale by 1/denom, one batched store.
"""
import os
import numpy as np
import ml_dtypes

import concourse.bass as bass
import concourse.mybir as mybir
import concourse.tile as tile
from concourse import bacc
from concourse.bass_utils import run_bass_kernel_spmd
from concourse.masks import make_identity

F32 = mybir.dt.float32
BF16 = mybir.dt.bfloat16
EXP = mybir.ActivationFunctionType.Exp
NPBF16 = ml_dtypes.bfloat16

B, S, D_IN, DK, DV = 4, 4096, 1024, 64, 64
SQ = S // 2            # 2048 query rows per core
NCH = D_IN // 128      # 8 d_in chunks
NKV = S // 128         # 32 kv chunks of 128
NQB = SQ // 512        # 4 query blocks of 512
KVG = 512              # kv granule column width (4 chunks)
NG = S // KVG          # 8 kv granules

_NC_CACHE = {}


def build_attention_nc():
    nc = bacc.Bacc()

    # host-preblocked granules: [g, p, c, s] = xT[c*128+p, g*GW+s]
    qt_ext = nc.declare_dram_parameter("qt", [4, 128, NCH, KVG], BF16, isOutput=False)
    kt_ext = nc.declare_dram_parameter("kt", [NG, 128, NCH, KVG], BF16, isOutput=False)
    vt_ext = nc.declare_dram_parameter("vt", [NG, 128, NCH, KVG], BF16, isOutput=False)
    # host-prepacked weights (bf16, Wq/Wk column-duplicated) and biases (dup)
    wq_ext = nc.declare_dram_parameter("wq", [128, NCH, 128], BF16, isOutput=False)
    wk_ext = nc.declare_dram_parameter("wk", [128, NCH, 128], BF16, isOutput=False)
    wv_ext = nc.declare_dram_parameter("wv", [128, NCH, DV], BF16, isOutput=False)
    bias_ext = nc.declare_dram_parameter("bias", [128, 3], F32, isOutput=False)
    out_ext = nc.declare_dram_parameter("out", [SQ, DV], F32, isOutput=True)

    with tile.TileContext(nc) as tc:
        with (
            tc.tile_pool(name="sg", bufs=1) as sg,
            tc.tile_pool(name="src", bufs=4) as srcp,
            tc.tile_pool(name="exp", bufs=6) as expp,
            tc.tile_pool(name="fin", bufs=2) as fin,
            tc.tile_pool(name="pp", bufs=2, space="PSUM") as pp,
            tc.tile_pool(name="sc", bufs=2, space="PSUM") as scp,
            tc.tile_pool(name="ot", bufs=2, space="PSUM") as otp,
        ):
            # ---- weights/biases first (small, prepacked), then streaming loads,
            # all on the single sync HWDGE ring so arrival order == issue order
            Wq = sg.tile([128, NCH, 128], BF16)
            Wk = sg.tile([128, NCH, 128], BF16)
            Wv = sg.tile([128, NCH, DV], BF16)
            biases = sg.tile([128, 3], F32)
            nc.sync.dma_start(out=Wq[:, :, :], in_=wq_ext[:, :, :])
            nc.sync.dma_start(out=biases[:, :], in_=bias_ext[:, :])

            src_tiles = {}

            def load(kind, idx):
                ext = {"q": qt_ext, "k": kt_ext, "v": vt_ext}[kind]
                t = srcp.tile(
                    [128, NCH, KVG], BF16, tag=f"{kind}src", bufs=3,
                    name=f"src_{kind}{idx}",
                )
                nc.sync.dma_start(out=t[:, :, :], in_=ext[idx])
                src_tiles[(kind, idx)] = t

            load("q", 0)
            nc.sync.dma_start(out=Wk[:, :, :], in_=wk_ext[:, :, :])
            load("k", 0)
            nc.sync.dma_start(out=Wv[:, :, :], in_=wv_ext[:, :, :])
            load("v", 0)
            load("q", 1)
            load("q", 2)
            load("q", 3)
            for g in range(1, NG):
                load("k", g)
                load("v", g)

            # ---- constants
            identb = sg.tile([128, 128], BF16)
            make_identity(nc, identb[:, :])
            identf = sg.tile([128, 128], F32)
            make_identity(nc, identf[:, :])
            ones = sg.tile([128, 1], BF16)
            nc.vector.memset(ones[:, :], 1.0)
            bqd = biases[:, 0:1]
            bkd = biases[:, 1:2]
            bvd = biases[0:64, 2:3]

            # projected tensors
            qTd = sg.tile([128, SQ], BF16)   # rows 0-63 = qT, 64-127 = copy
            kTd = sg.tile([128, S], BF16)    # rows 0-63 = kT, 64-127 = copy
            vT = sg.tile([64, S], BF16)      # [dv, kv]
            v1 = sg.tile([128, NKV, DV], BF16)  # v natural per chunk
            dacc = sg.tile([128, NQB, 1024], BF16)  # partial softmax denominators

            # prime PE clock and keep HAM warm while first loads land
            prime_ps = pp.tile([128, 128], BF16, tag="pp")
            for _ in range(8):
                nc.tensor.transpose(prime_ps[:, :], identb[:, :], identb[:, :])

            # PV accumulators: one bank per qb pair, col-tiled M=64 each
            otAB = otp.tile([128, 512], F32, tag="ot", name="otAB")
            otCD = otp.tile([128, 512], F32, tag="ot", name="otCD")

            def proj_mms(kind, idx):
                """Projection matmuls for one 512-col granule into a pp tile."""
                src = src_tiles.pop((kind, idx))
                W = {"q": Wq, "k": Wk, "v": Wv}[kind]
                mdim = 128 if kind != "v" else 64
                ps = pp.tile([128, 512], F32, tag="pp", name=f"pp_{kind}{idx}")
                for c in range(NCH):
                    nc.tensor.matmul(
                        ps[0:mdim, :],
                        W[:, c, 0:mdim],
                        src[:, c, :],
                        start=(c == 0),
                        stop=(c == NCH - 1),
                    )
                return ps

            def evict(kind, idx, ps):
                """Bias-add eviction, emitted a few steps after the matmuls so
                it does not head-of-line block ready exps in the Act queue."""
                col0 = KVG * idx
                if kind == "q":
                    nc.scalar.add(qTd[:, col0 : col0 + 512], ps[:, :], bqd)
                elif kind == "k":
                    nc.scalar.add(kTd[:, col0 : col0 + 512], ps[:, :], bkd)
                else:
                    nc.scalar.add(vT[:, col0 : col0 + 512], ps[0:64, :], bvd)

            def project(kind, idx):
                evict(kind, idx, proj_mms(kind, idx))

            def vflip(c):
                """vT chunk c -> v1[:, c, :] (natural [kv, dv])."""
                ps = pp.tile([128, DV], BF16, tag="pp", name=f"vf{c}")
                nc.tensor.transpose(
                    ps[:, :], vT[:, 128 * c : 128 * (c + 1)], identb[0:64, 0:64]
                )
                nc.vector.tensor_copy(v1[:, c, :], ps[:, :])

            def attn_pair(p, step):
                """Scores+exp+denom+PV for chunk pair (2p, 2p+1), all qb."""
                exs = {}
                for qb in range(NQB):
                    sps = scp.tile([128, 1024], F32, tag="sc", name=f"sc{p}_{qb}")
                    for j in range(2):
                        c = 2 * p + j
                        # alternate row-half per slot so the next slot's
                        # LDWEIGHTS targets the idle row group and pulls ahead
                        lo = 64 * ((j + qb) % 2)
                        hi = lo + 64
                        nc.tensor.matmul(
                            sps[:, 512 * j : 512 * j + 512],
                            kTd[lo:hi, 128 * c : 128 * (c + 1)],
                            qTd[lo:hi, 512 * qb : 512 * qb + 512],
                            start=True,
                            stop=True,
                        )
                    ex = expp.tile([128, 1024], BF16, tag="ex", name=f"ex{p}_{qb}")
                    nc.scalar.activation(out=ex[:, :], in_=sps[:, :], func=EXP, scale=0.125)
                    exs[qb] = ex
                    eng = nc.vector if qb < 2 else nc.gpsimd
                    if p == 0:
                        eng.tensor_copy(dacc[:, qb, :], ex[:, :])
                    else:
                        eng.tensor_add(dacc[:, qb, :], dacc[:, qb, :], ex[:, :])
                    if qb == 1:
                        for j in range(2):
                            c = 2 * p + j
                            for half, qa in ((0, 0), (64, 1)):
                                nc.tensor.matmul(
                                    otAB[half : half + 64, :],
                                    v1[:, c, :],
                                    exs[qa][:, 512 * j : 512 * j + 512],
                                    start=(c == 0),
                                    stop=(c == NKV - 1),
                                )
                    step()
                for j in range(2):
                    c = 2 * p + j
                    for half, qa in ((0, 2), (64, 3)):
                        nc.tensor.matmul(
                            otCD[half : half + 64, :],
                            v1[:, c, :],
                            exs[qa][:, 512 * j : 512 * j + 512],
                            start=(c == 0),
                            stop=(c == NKV - 1),
                        )

            # ---- prologue projections
            project("q", 0)
            project("k", 0)
            project("v", 0)
            for c in range(4):
                vflip(c)
            project("q", 1)

            # ---- main streaming loop over kv granules
            for g in range(NG):
                work = []
                if g == 0:
                    work = [lambda: project("q", 2), lambda: project("q", 3)]
                if g + 1 < NG:
                    work += [
                        lambda g=g: project("k", g + 1),
                        lambda g=g: project("v", g + 1),
                    ] + [
                        (lambda g=g, i=i: vflip(4 * (g + 1) + i)) for i in range(4)
                    ]
                it = iter(work)

                def step(it=it):
                    nxt = next(it, None)
                    if nxt is not None:
                        nxt()

                attn_pair(2 * g, step)
                attn_pair(2 * g + 1, step)
                step()
                step()

            # ---- epilogue: denominators, normalize, store
            dn = pp.tile([128, 16], F32, tag="pp", name="dn")
            for qb in range(NQB):
                for t in range(4):
                    for h in range(2):
                        nc.tensor.matmul(
                            dn[:, 4 * qb + t : 4 * qb + t + 1],
                            dacc[:, qb, 512 * h + 128 * t : 512 * h + 128 * t + 128],
                            ones[:, :],
                            start=(h == 0),
                            stop=(h == 1),
                        )
            rd = fin.tile([128, 16], F32, tag="rd")
            nc.vector.reciprocal(rd[:, :], dn[:, :])

            osbAB = fin.tile([128, 512], F32, tag="osb", name="osbAB")
            osbCD = fin.tile([128, 512], F32, tag="osb", name="osbCD")
            nc.vector.tensor_copy(osbAB[:, :], otAB[:, :])
            nc.scalar.copy(osbCD[:, :], otCD[:, :])

            stage = sg.tile([128, 16, DV], F32)
            for pair, osb in ((0, osbAB), (1, osbCD)):
                for t in range(4):
                    tp = pp.tile([128, 128], F32, tag="pp", name=f"tp{pair}_{t}")
                    nc.tensor.transpose(
                        tp[:, :], osb[:, 128 * t : 128 * t + 128], identf[:, :]
                    )
                    for h in range(2):
                        qb = 2 * pair + h
                        if h == 0:
                            nc.scalar.activation(
                                out=stage[:, 4 * qb + t, :],
                                in_=tp[:, 0:64],
                                func=mybir.ActivationFunctionType.Identity,
                                scale=rd[:, 4 * qb + t : 4 * qb + t + 1],
                            )
                        else:
                            nc.vector.tensor_scalar_mul(
                                stage[:, 4 * qb + t, :],
                                tp[:, 64 : 128],
                                rd[:, 4 * qb + t : 4 * qb + t + 1],
                            )
            nc.sync.dma_start(
                out=out_ext.rearrange("(b p) n -> p b n", p=128), in_=stage[:, :, :]
            )

    nc.compile()
    return nc


def _get_nc():
    if "nc" not in _NC_CACHE:
        _NC_CACHE["nc"] = build_attention_nc()
    return _NC_CACHE["nc"]


def kernel(query, key, value, Wq, bq, Wk, bk, Wv, bv):
    query = np.asarray(query, dtype=np.float32)
    key = np.asarray(key, dtype=np.float32)
    value = np.asarray(value, dtype=np.float32)
    wq = np.ascontiguousarray(np.asarray(Wq, np.float32))
    wk = np.ascontiguousarray(np.asarray(Wk, np.float32))
    wv = np.ascontiguousarray(np.asarray(Wv, np.float32))
    bq_ = np.ascontiguousarray(np.asarray(bq, np.float32))
    bk_ = np.ascontiguousarray(np.asarray(bk, np.float32))
    bv_ = np.ascontiguousarray(np.asarray(bv, np.float32))

    def preblock(x, gw):
        # x: [rows, 1024] f32 -> [rows/gw, 128, 8, gw] bf16 with
        # out[g, p, c, s] = x[g*gw + s, c*128 + p]
        ng = x.shape[0] // gw
        return np.ascontiguousarray(
            x.astype(NPBF16).reshape(ng, gw, NCH, 128).transpose(0, 3, 2, 1)
        )

    def packw(w, dup):
        # [1024, 64] f32 -> [128, 8, 128 or 64] bf16, [p, c, :] = w[c*128+p, :]
        w8 = w.astype(NPBF16).reshape(NCH, 128, 64).transpose(1, 0, 2)
        if dup:
            w8 = np.concatenate([w8, w8], axis=2)
        return np.ascontiguousarray(w8)

    wqp = packw(wq, True)
    wkp = packw(wk, True)
    wvp = packw(wv, False)
    biasp = np.ascontiguousarray(
        np.stack(
            [np.tile(bq_, 2), np.tile(bk_, 2), np.tile(bv_, 2)], axis=1
        ).astype(np.float32)
    )

    in_maps = []
    for b in range(B):
        ktb = preblock(key[b], KVG)
        vtb = preblock(value[b], KVG)
        for h in range(2):
            qtb = preblock(query[b, h * SQ : (h + 1) * SQ], KVG)
            in_maps.append(
                {
                    "qt": qtb, "kt": ktb, "vt": vtb,
                    "wq": wqp, "wk": wkp, "wv": wvp, "bias": biasp,
                }
            )

    nc = _get_nc()
    trace = bool(int(os.environ.get("BASS_KERNEL_TRACE", "0")))
    res = run_bass_kernel_spmd(nc, in_maps, core_ids=list(range(8)), trace=trace)
    _NC_CACHE["last_results"] = res

    out = np.empty((B, S, DV), np.float32)
    for b in range(B):
        for h in range(2):
            out[b, h * SQ : (h + 1) * SQ] = res.results[2 * b + h]["out"]
    return out
